# revision 1
# baseline (speedup 1.0000x reference)
"""BayesianLinear (y = x @ (mu + softplus(rho) * eps).T + bias) on 8 TRN2 cores.

Column-parallel sharding: each core owns OUT_F/8 = 512 output features.

Host-side prep is pure layout/precision staging (no reference math):
  - x is cast to bf16 and pre-tiled into the SBUF layout the TensorEngine
    needs for its stationary operand: x_t[bt, pi, po, bi] = x[bt*128+bi,
    po*128+pi], so each 128-row batch tile is one contiguous 1 MiB DMA.
  - weight_mu/rho/eps shards are transposed to [in_f, o_sh], tiled per
    128-row K-block, and PACKED into one bf16-typed tensor per K-block
    (mu bf16 | eps bf16 | rho fp16-bits) so W^T construction costs a
    single 384 KiB DMA per K-block. mu/eps ship as bf16 (their info is
    rounded into the bf16 W anyway); rho ships as fp16 because softplus
    amplifies its quantization ~3x and fp16 keeps that negligible.

Device per core:
  1. For each K-block k (32): one packed param DMA (GPSIMD SWDGE queue),
     softplus(rho) = Ln(1 + Exp(rho)) on ACT (no Softplus LUT on TRN2;
     Exp and Ln share one table), mul/add on DVE writing bf16 straight
     into the resident W^T tile [128, 32, 512]. No transpose on device.
  2. bias row = bias_mu + softplus(bias_rho) * bias_eps, built mid-
     construction (so its latency chain doesn't head-of-line block any
     engine queue), then broadcast across partitions with one K=1 matmul
     against a ones row — placed AFTER the first matmul group in PE
     program order, since the in-order PE stream would otherwise stall
     on the bias chain.
  3. First 8 batch tiles run k-interleaved across all 8 PSUM banks so the
     PE consumes W^T blocks no faster than construction produces them;
     their x tiles are loaded chunk-major (first K-quarter of all strips
     first — Tile tracks deps at AP-range granularity). Remaining 56
     tiles stream one PSUM bank each: one 1 MiB x DMA, 32 accumulating
     bf16 matmuls into PSUM [128, 512] fp32, DVE eviction fused with the
     bias add, DMA out.
"""

import numpy as np
import ml_dtypes

import concourse.bacc as bacc
import concourse.mybir as mybir
import concourse.tile as tile
from concourse.bass_utils import run_bass_kernel_spmd

BATCH = 8192
IN_F = 4096
OUT_F = 4096
N_CORES = 8
P = 128

_NC_CACHE = {}


def build_nc(batch=BATCH, in_f=IN_F, o_sh=OUT_F // N_CORES):
    KB = in_f // P  # K-blocks of 128 along the contraction dim
    BT = batch // P  # 128-row output tiles

    nc = bacc.Bacc(
        "TRN2",
        target_bir_lowering=False,
        debug=False,
        enable_asserts=False,
        num_devices=N_CORES,
    )
    bf16 = mybir.dt.bfloat16
    f16 = mybir.dt.float16
    f32 = mybir.dt.float32

    x = nc.declare_dram_parameter("x_t", [BT, P, KB, P], bf16, isOutput=False)
    K2 = 2 if KB % 2 == 0 else 1  # K-blocks per construction step
    wpk = nc.declare_dram_parameter(
        "wpk_t", [KB // K2, P, K2, 3 * o_sh], bf16, isOutput=False
    )
    bmu = nc.declare_dram_parameter("bias_mu", [1, o_sh], f32, isOutput=False)
    brho = nc.declare_dram_parameter("bias_rho", [1, o_sh], f32, isOutput=False)
    beps = nc.declare_dram_parameter("bias_eps", [1, o_sh], f32, isOutput=False)
    y = nc.declare_dram_parameter("y", [batch, o_sh], f32, isOutput=True)

    act_exp = mybir.ActivationFunctionType.Exp
    act_ln = mybir.ActivationFunctionType.Ln

    with tile.TileContext(nc) as tc:
        with (
            tc.tile_pool(name="const", bufs=1) as const,
            tc.tile_pool(name="wcons", bufs=4) as wcons,
            tc.tile_pool(name="xin", bufs=10) as xin,
            tc.tile_pool(name="yout", bufs=4) as yout,
            tc.tile_pool(name="psum", bufs=7, space="PSUM") as psum_pool,
            tc.tile_pool(name="bpsum", bufs=1, space="PSUM") as bias_psum,
        ):
            bias_sb = const.tile([P, o_sh], f32, tag="bias_sb")
            bias_bf = const.tile([1, o_sh], bf16, tag="bias_bf")
            ones = const.tile([1, P], bf16, tag="ones")
            nc.vector.memset(ones[:], 1.0)
            wones = const.tile([1, o_sh], bf16, tag="wones")
            nc.vector.memset(wones[:], 1.0)

            # PE warmup: dummy K=1 matmuls with no DMA deps keep the PE
            # HAM-busy through the first W^T block's latency chain, so the
            # real matmul stream starts at the warm 2.4 GHz clock instead
            # of paying ~12us of cold-clock inflation plus an idle gap.
            warm_ps = bias_psum.tile([P, o_sh], f32, tag="bias_ps", name="warm_ps")
            for w in range(28):
                nc.tensor.matmul(warm_ps[:], lhsT=ones[:], rhs=wones[:])

            def emit_bias_row():
                b_mu = const.tile([1, o_sh], f32, tag="b_mu")
                b_rho = const.tile([1, o_sh], f32, tag="b_rho")
                b_eps = const.tile([1, o_sh], f32, tag="b_eps")
                nc.gpsimd.dma_start(out=b_mu[:], in_=bmu[:])
                nc.gpsimd.dma_start(out=b_rho[:], in_=brho[:])
                nc.gpsimd.dma_start(out=b_eps[:], in_=beps[:])
                b_sp = const.tile([1, o_sh], f32, tag="b_sp")
                nc.scalar.activation(b_sp[:], b_rho[:], act_exp)
                nc.scalar.activation(b_sp[:], b_sp[:], act_ln, bias=1.0)
                nc.vector.tensor_mul(out=b_sp[:], in0=b_sp[:], in1=b_eps[:])
                nc.vector.tensor_add(out=bias_bf[:], in0=b_sp[:], in1=b_mu[:])

            # ---- W^T constructed in place, one packed DMA per K2 blocks
            # (pairing K-blocks halves the per-op ACT/DVE fixed overhead and
            # the DMA trigger count, so production outruns the PE's warm
            # consumption during the overlap group).
            WT = const.tile([P, KB, o_sh], bf16, tag="WT")
            for k2 in range(KB // K2):
                pk = wcons.tile([P, K2, 3 * o_sh], bf16, tag="pk")
                nc.gpsimd.dma_start(out=pk[:], in_=wpk[k2])
                mu_t = pk[:, :, 0:o_sh]
                eps_t = pk[:, :, o_sh : 2 * o_sh]
                rho_t = pk[:, :, 2 * o_sh : 3 * o_sh].bitcast(f16)
                sp_t = wcons.tile([P, K2, o_sh], f32, tag="sp")
                nc.scalar.activation(sp_t[:], rho_t[:], act_exp)
                nc.scalar.activation(sp_t[:], sp_t[:], act_ln, bias=1.0)
                nc.vector.tensor_mul(out=sp_t[:], in0=sp_t[:], in1=eps_t[:])
                nc.vector.tensor_add(
                    out=WT[:, k2 * K2 : (k2 + 1) * K2, :], in0=sp_t[:], in1=mu_t[:]
                )
                if k2 == min(1, KB // K2 - 1):
                    emit_bias_row()

            def body_tail(ps, bt):
                y_sb = yout.tile([P, o_sh], f32, tag="y_sb")
                nc.vector.tensor_add(out=y_sb[:], in0=ps[:], in1=bias_sb[:])
                nc.sync.dma_start(out=y[bt * P : (bt + 1) * P, :], in_=y_sb[:])

            # ---- first GROUP tiles run k-interleaved across PSUM banks so
            # the PE consumes W^T blocks no faster than construction makes
            # them — the weight-construction latency hides under matmuls.
            GROUP = min(7, BT)
            xts = []
            pss = []
            for bt in range(GROUP):
                xT = xin.tile([P, KB, P], bf16, tag="xT", name=f"xT_g{bt}")
                xts.append(xT)
                ps = psum_pool.tile([P, o_sh], f32, tag="ps", name=f"ps_g{bt}")
                pss.append(ps)
            # chunk-major strip loads: the first K-quarter of every strip
            # lands before any second quarter, so the k=0 matmul batch isn't
            # gated on the last strip's full 1 MiB transfer.
            CH = 4 if KB % 4 == 0 else 1
            for c in range(CH):
                ks = slice(c * (KB // CH), (c + 1) * (KB // CH))
                for i in range(GROUP):
                    nc.sync.dma_start(out=xts[i][:, ks, :], in_=x[i, :, ks, :])
            for k in range(KB):
                for i in range(GROUP):
                    nc.tensor.matmul(
                        pss[i][:],
                        lhsT=xts[i][:, k, :],
                        rhs=WT[:, k, :],
                        start=(k == 0),
                        stop=(k == KB - 1),
                    )
                if k == min(8, KB - 1):
                    # bias broadcast: [128, o_sh] = ones.T @ bias_bf. Mid-
                    # stream (bias_bf is ready by now) so bias_sb exists
                    # before the first group eviction — the in-order PE
                    # stream must not head-of-line block on the bias chain.
                    bias_ps = bias_psum.tile(
                        [P, o_sh], f32, tag="bias_ps", name="bias_ps"
                    )
                    nc.tensor.matmul(bias_ps[:], lhsT=ones[:], rhs=bias_bf[:])
                    nc.vector.tensor_copy(out=bias_sb[:], in_=bias_ps[:])

            for i in range(GROUP):
                body_tail(pss[i], i)

            # ---- remaining tiles stream one PSUM bank each
            for bt in range(GROUP, BT):
                xT = xin.tile([P, KB, P], bf16, tag="xT")
                nc.sync.dma_start(out=xT[:], in_=x[bt])
                ps = psum_pool.tile([P, o_sh], f32, tag="ps")
                for k in range(KB):
                    nc.tensor.matmul(
                        ps[:],
                        lhsT=xT[:, k, :],
                        rhs=WT[:, k, :],
                        start=(k == 0),
                        stop=(k == KB - 1),
                    )
                body_tail(ps, bt)

    # Skip bacc's pre-placed InstLoadActFuncSet: on large graphs walrus's
    # parallel-pass fork can separate the hoisted load from its activations
    # ("No Act func set exist for this instruction"); walrus's own lower_act
    # placement handles forked subgraphs correctly.
    nc.insert_act_table_loads = lambda: None
    nc.compile()
    return nc


def _prep_x(x):
    """[batch, in_f] fp32 -> bf16 tiled [BT, 128, KB, 128] with
    x_t[bt, pi, po, bi] = x[bt*128 + bi, po*128 + pi]."""
    batch, in_f = x.shape
    xb = x.astype(ml_dtypes.bfloat16)
    xb = xb.reshape(batch // P, P, in_f // P, P)  # [bt, bi, po, pi]
    return np.ascontiguousarray(xb.transpose(0, 3, 2, 1))  # [bt, pi, po, bi]


def _tile_w(w, dtype):
    """[o_sh, in_f] -> tiled [KB, 128, o_sh] with w_t[k, pi, o] = w[o, k*128 + pi]."""
    o_sh, in_f = w.shape
    return np.ascontiguousarray(w.T.reshape(in_f // P, P, o_sh)).astype(dtype)


def _prep_wpk(wmu, wrho, weps):
    """Pack mu (bf16), eps (bf16), rho (fp16 bits viewed as bf16) into one
    bf16-typed [KB/K2, 128, K2, 3*o_sh] tensor — one DMA per K2 K-blocks."""
    mu = _tile_w(wmu, ml_dtypes.bfloat16)
    eps = _tile_w(weps, ml_dtypes.bfloat16)
    rho = _tile_w(wrho, np.float16).view(ml_dtypes.bfloat16)
    pk = np.concatenate([mu, eps, rho], axis=2)  # [KB, P, 3*o_sh]
    kb, p, f = pk.shape
    k2 = 2 if kb % 2 == 0 else 1
    pk = pk.reshape(kb // k2, k2, p, f).transpose(0, 2, 1, 3)
    return np.ascontiguousarray(pk)


def make_in_maps(x, weight_mu, weight_rho, bias_mu, bias_rho, weight_eps, bias_eps):
    o_sh = OUT_F // N_CORES
    x_t = _prep_x(np.asarray(x, dtype=np.float32))
    wmu = np.asarray(weight_mu, dtype=np.float32)
    wrho = np.asarray(weight_rho, dtype=np.float32)
    weps = np.asarray(weight_eps, dtype=np.float32)
    bmu = np.asarray(bias_mu, dtype=np.float32).reshape(1, -1)
    brho = np.asarray(bias_rho, dtype=np.float32).reshape(1, -1)
    beps = np.asarray(bias_eps, dtype=np.float32).reshape(1, -1)

    in_maps = []
    for c in range(N_CORES):
        rs = slice(c * o_sh, (c + 1) * o_sh)
        in_maps.append(
            {
                "x_t": x_t,
                "wpk_t": _prep_wpk(wmu[rs], wrho[rs], weps[rs]),
                "bias_mu": np.ascontiguousarray(bmu[:, rs]),
                "bias_rho": np.ascontiguousarray(brho[:, rs]),
                "bias_eps": np.ascontiguousarray(beps[:, rs]),
            }
        )
    return in_maps


def kernel(x, weight_mu, weight_rho, bias_mu, bias_rho, weight_eps, bias_eps):
    o_sh = OUT_F // N_CORES
    key = (x.shape, o_sh)
    if key not in _NC_CACHE:
        _NC_CACHE[key] = build_nc(x.shape[0], x.shape[1], o_sh)
    nc = _NC_CACHE[key]

    in_maps = make_in_maps(
        x, weight_mu, weight_rho, bias_mu, bias_rho, weight_eps, bias_eps
    )
    res = run_bass_kernel_spmd(nc, in_maps, core_ids=list(range(N_CORES)))
    return np.concatenate([res.results[c]["y"] for c in range(N_CORES)], axis=1)



# revision 2
# speedup vs baseline: 1.0028x; 1.0028x over previous
"""BayesianLinear (y = x @ (mu + softplus(rho) * eps).T + bias) on 8 TRN2 cores.

Column-parallel sharding: each core owns OUT_F/8 = 512 output features.

Host-side prep is pure layout/precision staging (no reference math):
  - x is cast to bf16 and pre-tiled into the SBUF layout the TensorEngine
    needs for its stationary operand: x_t[bt, pi, po, bi] = x[bt*128+bi,
    po*128+pi], so each 128-row batch tile is one contiguous 1 MiB DMA.
  - weight params are transposed to [in_f, o_sh], tiled per 128-row
    K-block, and PACKED into one uint8 tensor [128, KB, 2048]:
    eps bf16 (1024 B) | mu int8 (512 B) | rho uint8 (512 B) per block.
    mu/rho ship as affine-quantized 8-bit codes (scales in a tiny qp
    tensor): uniform 8-bit beats fp8 ~3x in rms error for Gaussian data
    and halves the packed-weight HBM traffic (12.6 -> 8.4 MiB/core) --
    the construction phase is HBM-bound, not compute-bound.

Device per core:
  1. Construction units (2 single K-blocks first for a short critical
     path, then 15 pairs): one packed DMA (GPSIMD SWDGE queue), then
     softplus(rho) = Ln(1 + Exp(rho)) on ACT -- the uint8 rho dequant
     rides Exp's free affine (out = f(scale*in + bias), scale/bias as
     per-partition APs from qp). DVE: mul by eps (bf16 2x mode), then
     one fused scalar_tensor_tensor (mu_i8 * mu_scale) + sp_eps writing
     bf16 straight into the resident W^T tile [128, 32, 512].
  2. bias row = bias_mu + softplus(bias_rho) * bias_eps (fp32, tiny),
     broadcast across partitions with one K=1 matmul against a ones row
     mid-wave so the in-order PE stream never head-of-line blocks on it.
  3. First 7 batch tiles run as a k-WAVEFRONT across the 7 PSUM banks:
     wave w issues tile i's matmul for k = w - i. Tile 0's first matmul
     only needs W^T block 0 + one 256 KiB x chunk, so real work starts
     ~12 us in (vs ~27 us for the k-major group), and the PE consumes
     W^T blocks no faster than construction produces them. x chunks are
     DMA'd in need-order with a 10-wave lookahead so the packed-weight
     DMAs get their required HBM share.
  4. A short PE warmup (dummy K=1 matmuls, no DMA deps) bridges the
     framework preamble to first-data-ready so the HAM clock gate is
     already at 8/8 when the real stream starts.
  5. Remaining 56 tiles stream one PSUM bank each: one 1 MiB x DMA, 32
     accumulating bf16 matmuls into PSUM [128, 512] fp32, DVE eviction
     fused with the bias add, DMA out.
"""

import numpy as np
import ml_dtypes

import concourse.bacc as bacc
import concourse.mybir as mybir
import concourse.tile as tile
from concourse.bass_utils import run_bass_kernel_spmd

BATCH = 8192
IN_F = 4096
OUT_F = 4096
N_CORES = 8
P = 128

_NC_CACHE = {}

PKB = 2048  # packed bytes per partition per K-block: eps 1024 | mu 512 | rho 512
WARM = 10  # PE warmup matmuls
LOOKAHEAD = 10  # waves of x-chunk DMA lookahead
CH = 4  # x chunks per strip (8 K-blocks each)


def build_nc(batch=BATCH, in_f=IN_F, o_sh=OUT_F // N_CORES):
    KB = in_f // P  # K-blocks of 128 along the contraction dim
    BT = batch // P  # 128-row output tiles

    nc = bacc.Bacc(
        "TRN2",
        target_bir_lowering=False,
        debug=False,
        enable_asserts=False,
        num_devices=N_CORES,
    )
    bf16 = mybir.dt.bfloat16
    f32 = mybir.dt.float32
    u8 = mybir.dt.uint8
    i8 = mybir.dt.int8

    x = nc.declare_dram_parameter("x_t", [BT, P, KB, P], bf16, isOutput=False)
    wpk = nc.declare_dram_parameter("wpk_t", [P, KB, PKB], u8, isOutput=False)
    qp = nc.declare_dram_parameter("qp", [P, 4], f32, isOutput=False)
    bmu = nc.declare_dram_parameter("bias_mu", [1, o_sh], f32, isOutput=False)
    brho = nc.declare_dram_parameter("bias_rho", [1, o_sh], f32, isOutput=False)
    beps = nc.declare_dram_parameter("bias_eps", [1, o_sh], f32, isOutput=False)
    y = nc.declare_dram_parameter("y", [batch, o_sh], f32, isOutput=True)

    act_exp = mybir.ActivationFunctionType.Exp
    act_ln = mybir.ActivationFunctionType.Ln
    op_mult = mybir.AluOpType.mult
    op_add = mybir.AluOpType.add

    # construction units: two single K-blocks (short first-ready chain),
    # then pairs
    units = [(0, 1), (1, 1)]
    b = 2
    while b < KB:
        s = min(2, KB - b)
        units.append((b, s))
        b += s

    with tile.TileContext(nc) as tc:
        with (
            tc.tile_pool(name="const", bufs=1) as const,
            tc.tile_pool(name="wcons", bufs=4) as wcons,
            tc.tile_pool(name="xin", bufs=10) as xin,
            tc.tile_pool(name="yout", bufs=4) as yout,
            tc.tile_pool(name="psum", bufs=7, space="PSUM") as psum_pool,
            tc.tile_pool(name="bpsum", bufs=1, space="PSUM") as bias_psum,
        ):
            bias_sb = const.tile([P, o_sh], f32, tag="bias_sb")
            bias_bf = const.tile([1, o_sh], bf16, tag="bias_bf")
            ones = const.tile([1, P], bf16, tag="ones")
            nc.vector.memset(ones[:], 1.0)
            wones = const.tile([1, o_sh], bf16, tag="wones")
            nc.vector.memset(wones[:], 1.0)
            qp_sb = const.tile([P, 4], f32, tag="qp_sb")
            nc.sync.dma_start(out=qp_sb[:], in_=qp[:])
            rho_sc = qp_sb[:, 0:1]
            rho_min = qp_sb[:, 1:2]
            mu_sc = qp_sb[:, 2:3]

            # PE warmup: dummy K=1 matmuls with no DMA deps bridge the
            # ~6.5us framework preamble to first-data-ready (~12us) so
            # the HAM clock gate is 8/8 when the real stream starts.
            warm_ps = bias_psum.tile([P, o_sh], f32, tag="bias_ps", name="warm_ps")
            for w in range(WARM):
                nc.tensor.matmul(warm_ps[:], lhsT=ones[:], rhs=wones[:])

            def emit_bias_row():
                b_mu = const.tile([1, o_sh], f32, tag="b_mu")
                b_rho = const.tile([1, o_sh], f32, tag="b_rho")
                b_eps = const.tile([1, o_sh], f32, tag="b_eps")
                nc.sync.dma_start(out=b_mu[:], in_=bmu[:])
                nc.sync.dma_start(out=b_rho[:], in_=brho[:])
                nc.sync.dma_start(out=b_eps[:], in_=beps[:])
                b_sp = const.tile([1, o_sh], f32, tag="b_sp")
                nc.scalar.activation(b_sp[:], b_rho[:], act_exp)
                nc.scalar.activation(b_sp[:], b_sp[:], act_ln, bias=1.0)
                nc.vector.tensor_mul(out=b_sp[:], in0=b_sp[:], in1=b_eps[:])
                nc.vector.tensor_add(out=bias_bf[:], in0=b_sp[:], in1=b_mu[:])

            # ---- W^T constructed in place. Per unit: one packed DMA,
            # Exp with fused uint8-rho dequant, Ln, eps-mul (bf16 2x),
            # fused (mu_i8 * scale) + sp_eps -> bf16 W^T block.
            WT = const.tile([P, KB, o_sh], bf16, tag="WT")
            for ui, (ub, us) in enumerate(units):
                pk = wcons.tile([P, us, PKB], u8, tag="pk")
                nc.gpsimd.dma_start(out=pk[:], in_=wpk[:, ub : ub + us, :])
                eps_v = pk[:, :, 0 : 2 * o_sh].bitcast(bf16)
                mu_v = pk[:, :, 2 * o_sh : 3 * o_sh].bitcast(i8)
                rho_v = pk[:, :, 3 * o_sh : 4 * o_sh]
                sp = wcons.tile([P, us, o_sh], bf16, tag="sp")
                nc.scalar.activation(sp[:], rho_v[:], act_exp, bias=rho_min, scale=rho_sc)
                nc.scalar.activation(sp[:], sp[:], act_ln, bias=1.0)
                nc.vector.tensor_mul(out=sp[:], in0=sp[:], in1=eps_v[:])
                nc.vector.scalar_tensor_tensor(
                    out=WT[:, ub : ub + us, :],
                    in0=mu_v[:],
                    scalar=mu_sc,
                    in1=sp[:],
                    op0=op_mult,
                    op1=op_add,
                )
                if ui == 2:
                    emit_bias_row()

            def body_tail(ps, bt):
                y_sb = yout.tile([P, o_sh], f32, tag="y_sb")
                nc.vector.tensor_add(out=y_sb[:], in0=ps[:], in1=bias_sb[:])
                nc.sync.dma_start(out=y[bt * P : (bt + 1) * P, :], in_=y_sb[:])

            # ---- first GROUP tiles run as a k-wavefront across PSUM
            # banks: wave w = tile i's matmul for k = w - i. Tile 0's
            # k=0 matmul needs only W^T block 0 + one x chunk.
            GROUP = min(7, BT)
            KC = KB // CH  # K-blocks per x chunk
            xts = []
            pss = []
            for bt in range(GROUP):
                xT = xin.tile([P, KB, P], bf16, tag="xT", name=f"xT_g{bt}")
                xts.append(xT)
                ps = psum_pool.tile([P, o_sh], f32, tag="ps", name=f"ps_g{bt}")
                pss.append(ps)

            # x chunk (i, c) is first read at wave i + c*KC; DMA in need
            # order with LOOKAHEAD waves of headroom so the packed-weight
            # DMAs keep their HBM share.
            chunks = sorted(
                ((i + c * KC, i, c) for i in range(GROUP) for c in range(CH))
            )

            def issue_chunks_through(wave):
                while chunks and chunks[0][0] <= wave:
                    _, i, c = chunks.pop(0)
                    ks = slice(c * KC, (c + 1) * KC)
                    nc.sync.dma_start(out=xts[i][:, ks, :], in_=x[i, :, ks, :])

            issue_chunks_through(LOOKAHEAD - 1)
            NWAVE = KB + GROUP - 1
            for w in range(NWAVE):
                issue_chunks_through(w + LOOKAHEAD)
                for i in range(GROUP):
                    k = w - i
                    if 0 <= k < KB:
                        nc.tensor.matmul(
                            pss[i][:],
                            lhsT=xts[i][:, k, :],
                            rhs=WT[:, k, :],
                            start=(k == 0),
                            stop=(k == KB - 1),
                        )
                if w == 12:
                    # bias broadcast: [128, o_sh] = ones.T @ bias_bf.
                    # Mid-stream so the in-order PE queue never blocks
                    # on the bias chain; ready long before 1st eviction.
                    bias_ps = bias_psum.tile(
                        [P, o_sh], f32, tag="bias_ps", name="bias_ps"
                    )
                    nc.tensor.matmul(bias_ps[:], lhsT=ones[:], rhs=bias_bf[:])
                    nc.vector.tensor_copy(out=bias_sb[:], in_=bias_ps[:])
                gi = w - (KB - 1)
                if 0 <= gi < GROUP:
                    body_tail(pss[gi], gi)

            # ---- remaining tiles stream one PSUM bank each
            for bt in range(GROUP, BT):
                xT = xin.tile([P, KB, P], bf16, tag="xT")
                nc.sync.dma_start(out=xT[:], in_=x[bt])
                ps = psum_pool.tile([P, o_sh], f32, tag="ps")
                for k in range(KB):
                    nc.tensor.matmul(
                        ps[:],
                        lhsT=xT[:, k, :],
                        rhs=WT[:, k, :],
                        start=(k == 0),
                        stop=(k == KB - 1),
                    )
                body_tail(ps, bt)

    # Skip bacc's pre-placed InstLoadActFuncSet: on large graphs walrus's
    # parallel-pass fork can separate the hoisted load from its activations
    # ("No Act func set exist for this instruction"); walrus's own lower_act
    # placement handles forked subgraphs correctly.
    nc.insert_act_table_loads = lambda: None
    nc.compile()
    return nc


def _prep_x(x):
    """[batch, in_f] fp32 -> bf16 tiled [BT, 128, KB, 128] with
    x_t[bt, pi, po, bi] = x[bt*128 + bi, po*128 + pi]."""
    batch, in_f = x.shape
    xb = x.astype(ml_dtypes.bfloat16)
    xb = xb.reshape(batch // P, P, in_f // P, P)  # [bt, bi, po, pi]
    return np.ascontiguousarray(xb.transpose(0, 3, 2, 1))  # [bt, pi, po, bi]


def _tile_w(w):
    """[o_sh, in_f] -> tiled [KB, 128, o_sh] with w_t[k, pi, o] = w[o, k*128 + pi]."""
    o_sh, in_f = w.shape
    return np.ascontiguousarray(w.T.reshape(in_f // P, P, o_sh))


def _prep_wpk(wmu, wrho, weps):
    """Pack eps (bf16 bytes), mu (int8 codes), rho (uint8 codes) into one
    uint8 [128, KB, 2048] tensor + the fp32 quant params [128, 4]."""
    eps_t = _tile_w(weps).astype(ml_dtypes.bfloat16)  # [KB, P, o]
    mu_t = _tile_w(wmu)
    rho_t = _tile_w(wrho)

    mu_sc = max(float(np.abs(mu_t).max()) / 127.0, 1e-30)
    mu_c = np.clip(np.round(mu_t / mu_sc), -127, 127).astype(np.int8)

    rmin = float(rho_t.min())
    rmax = float(rho_t.max())
    rho_sc = max((rmax - rmin) / 255.0, 1e-30)
    rho_c = np.clip(np.round((rho_t - rmin) / rho_sc), 0, 255).astype(np.uint8)

    kb, p, o = mu_t.shape
    pk = np.concatenate(
        [
            eps_t.view(np.uint8).reshape(kb, p, 2 * o),
            mu_c.view(np.uint8),
            rho_c,
        ],
        axis=2,
    )  # [KB, P, 4*o]
    qp = np.broadcast_to(
        np.array([rho_sc, rmin, mu_sc, 0.0], np.float32), (P, 4)
    ).copy()
    return np.ascontiguousarray(pk.transpose(1, 0, 2)), qp


def make_in_maps(x, weight_mu, weight_rho, bias_mu, bias_rho, weight_eps, bias_eps):
    o_sh = OUT_F // N_CORES
    x_t = _prep_x(np.asarray(x, dtype=np.float32))
    wmu = np.asarray(weight_mu, dtype=np.float32)
    wrho = np.asarray(weight_rho, dtype=np.float32)
    weps = np.asarray(weight_eps, dtype=np.float32)
    bmu = np.asarray(bias_mu, dtype=np.float32).reshape(1, -1)
    brho = np.asarray(bias_rho, dtype=np.float32).reshape(1, -1)
    beps = np.asarray(bias_eps, dtype=np.float32).reshape(1, -1)

    in_maps = []
    for c in range(N_CORES):
        rs = slice(c * o_sh, (c + 1) * o_sh)
        wpk, qp = _prep_wpk(wmu[rs], wrho[rs], weps[rs])
        in_maps.append(
            {
                "x_t": x_t,
                "wpk_t": wpk,
                "qp": qp,
                "bias_mu": np.ascontiguousarray(bmu[:, rs]),
                "bias_rho": np.ascontiguousarray(brho[:, rs]),
                "bias_eps": np.ascontiguousarray(beps[:, rs]),
            }
        )
    return in_maps


def kernel(x, weight_mu, weight_rho, bias_mu, bias_rho, weight_eps, bias_eps):
    o_sh = OUT_F // N_CORES
    key = (x.shape, o_sh)
    if key not in _NC_CACHE:
        _NC_CACHE[key] = build_nc(x.shape[0], x.shape[1], o_sh)
    nc = _NC_CACHE[key]

    in_maps = make_in_maps(
        x, weight_mu, weight_rho, bias_mu, bias_rho, weight_eps, bias_eps
    )
    res = run_bass_kernel_spmd(nc, in_maps, core_ids=list(range(N_CORES)))
    return np.concatenate([res.results[c]["y"] for c in range(N_CORES)], axis=1)


# revision 4
# speedup vs baseline: 1.2066x; 1.2032x over previous
"""BayesianLinear (y = x @ (mu + softplus(rho) * eps).T + bias) on 8 TRN2 cores.

Column-parallel sharding: each core owns OUT_F/8 = 512 output features.

Host-side prep is pure layout/precision staging (no reference math):
  - x is cast to bf16 and pre-tiled into the SBUF layout the TensorEngine
    needs for its stationary operand: x_t[bt, pi, po, bi] = x[bt*128+bi,
    po*128+pi], so each 128-row batch tile is one contiguous 1 MiB DMA.
  - weight params are transposed to [in_f, o_sh], tiled per 128-row
    K-block, and PACKED into one uint8 tensor [128, KB, 2048]:
    eps bf16 (1024 B) | mu int8 (512 B) | rho uint8 (512 B) per block.
    mu/rho ship as affine-quantized 8-bit codes (scales in a tiny qp
    tensor): uniform 8-bit beats fp8 ~3x in rms error for Gaussian data
    and halves the packed-weight HBM traffic (12.6 -> 8.4 MiB/core) --
    the construction phase is HBM-bound, not compute-bound.

Device per core:
  1. Construction units (2 single K-blocks first for a short critical
     path, then 15 pairs): one packed DMA (GPSIMD SWDGE queue), then
     softplus(rho) = Ln(1 + Exp(rho)) on ACT -- the uint8 rho dequant
     rides Exp's free affine (out = f(scale*in + bias), scale/bias as
     per-partition APs from qp). DVE: mul by eps (bf16 2x mode), then
     one fused scalar_tensor_tensor (mu_i8 * mu_scale) + sp_eps writing
     bf16 straight into the resident W^T tile [128, 32, 512].
  2. bias row = bias_mu + softplus(bias_rho) * bias_eps (fp32, tiny),
     broadcast across partitions with one K=1 matmul against a ones row
     mid-wave so the in-order PE stream never head-of-line blocks on it.
  3. First 7 batch tiles run as a k-WAVEFRONT across the 7 PSUM banks:
     wave w issues tile i's matmul for k = w - i. Tile 0's first matmul
     only needs W^T block 0 + one 256 KiB x chunk, so real work starts
     ~12 us in (vs ~27 us for the k-major group), and the PE consumes
     W^T blocks no faster than construction produces them. x chunks are
     DMA'd in need-order with a 10-wave lookahead so the packed-weight
     DMAs get their required HBM share.
  4. A short PE warmup (dummy K=1 matmuls, no DMA deps) bridges the
     framework preamble to first-data-ready so the HAM clock gate is
     already at 8/8 when the real stream starts.
  5. Remaining 56 tiles stream one PSUM bank each: one 1 MiB x DMA, 32
     accumulating bf16 matmuls into PSUM [128, 512] fp32, DVE eviction
     fused with the bias add, DMA out.
"""

import numpy as np
import ml_dtypes

import concourse.bacc as bacc
import concourse.mybir as mybir
import concourse.tile as tile
from concourse.bass_utils import run_bass_kernel_spmd

BATCH = 8192
IN_F = 4096
OUT_F = 4096
N_CORES = 8
P = 128

_NC_CACHE = {}

PKB = 2048  # packed bytes per partition per K-block: eps 1024 | mu 512 | rho 512
WARM = 10  # PE warmup matmuls
LOOKAHEAD = 10  # waves of x-chunk DMA lookahead
CH = 4  # x chunks per strip (8 K-blocks each)


def build_nc(batch=BATCH, in_f=IN_F, o_sh=OUT_F // N_CORES):
    KB = in_f // P  # K-blocks of 128 along the contraction dim
    BT = batch // P  # 128-row output tiles

    nc = bacc.Bacc(
        "TRN2",
        target_bir_lowering=False,
        debug=False,
        enable_asserts=False,
        num_devices=N_CORES,
    )
    bf16 = mybir.dt.bfloat16
    f32 = mybir.dt.float32
    u8 = mybir.dt.uint8
    i8 = mybir.dt.int8

    x = nc.declare_dram_parameter("x_t", [BT, P, KB, P], bf16, isOutput=False)
    wpk = nc.declare_dram_parameter("wpk_t", [P, KB, PKB], u8, isOutput=False)
    qp = nc.declare_dram_parameter("qp", [P, 4], f32, isOutput=False)
    bmu = nc.declare_dram_parameter("bias_mu", [1, o_sh], f32, isOutput=False)
    brho = nc.declare_dram_parameter("bias_rho", [1, o_sh], f32, isOutput=False)
    beps = nc.declare_dram_parameter("bias_eps", [1, o_sh], f32, isOutput=False)
    y = nc.declare_dram_parameter("y", [batch, o_sh], f32, isOutput=True)

    act_exp = mybir.ActivationFunctionType.Exp
    act_ln = mybir.ActivationFunctionType.Ln
    op_mult = mybir.AluOpType.mult
    op_add = mybir.AluOpType.add

    # construction units: two single K-blocks (short first-ready chain),
    # one pair, then quads — bigger ACT passes amortize the ~352-cycle
    # fixed cost; the serial ACT chain is the construction floor.
    units = [(0, 1), (1, 1), (2, 2)]
    b = 4
    while b < KB:
        s = min(4, KB - b)
        units.append((b, s))
        b += s

    with tile.TileContext(nc) as tc:
        with (
            tc.tile_pool(name="const", bufs=1) as const,
            tc.tile_pool(name="wcons", bufs=4) as wcons,
            # bufs=8: 7 live group strips + ONE streaming-prefetch slot.
            # More slots would let 1 MiB streaming prefetches flood the
            # DMA rings mid-group and starve the packed-weight DMAs the
            # ACT pipeline is waiting on (measured: wpk latency 11 us,
            # ACT blocked 27 us). Slots open progressively as group
            # strips retire, so streaming runway builds during the
            # group tail exactly when HBM frees up.
            tc.tile_pool(name="xin", bufs=8) as xin,
            tc.tile_pool(name="yout", bufs=4) as yout,
            tc.tile_pool(name="psum", bufs=7, space="PSUM") as psum_pool,
            tc.tile_pool(name="bpsum", bufs=1, space="PSUM") as bias_psum,
        ):
            bias_sb = const.tile([P, o_sh], f32, tag="bias_sb")
            bias_bf = const.tile([1, o_sh], bf16, tag="bias_bf")
            ones = const.tile([1, P], bf16, tag="ones")
            nc.vector.memset(ones[:], 1.0)
            wones = const.tile([1, o_sh], bf16, tag="wones")
            nc.vector.memset(wones[:], 1.0)
            qp_sb = const.tile([P, 4], f32, tag="qp_sb")
            nc.sync.dma_start(out=qp_sb[:], in_=qp[:])
            rho_sc = qp_sb[:, 0:1]
            rho_min = qp_sb[:, 1:2]
            mu_sc = qp_sb[:, 2:3]

            # PE warmup: dummy K=1 matmuls with no DMA deps bridge the
            # ~6.5us framework preamble to first-data-ready (~12us) so
            # the HAM clock gate is 8/8 when the real stream starts.
            warm_ps = bias_psum.tile([P, o_sh], f32, tag="bias_ps", name="warm_ps")
            for w in range(WARM):
                nc.tensor.matmul(warm_ps[:], lhsT=ones[:], rhs=wones[:])

            def emit_bias_row():
                b_mu = const.tile([1, o_sh], f32, tag="b_mu")
                b_rho = const.tile([1, o_sh], f32, tag="b_rho")
                b_eps = const.tile([1, o_sh], f32, tag="b_eps")
                nc.sync.dma_start(out=b_mu[:], in_=bmu[:])
                nc.sync.dma_start(out=b_rho[:], in_=brho[:])
                nc.sync.dma_start(out=b_eps[:], in_=beps[:])
                b_sp = const.tile([1, o_sh], f32, tag="b_sp")
                nc.scalar.activation(b_sp[:], b_rho[:], act_exp)
                nc.scalar.activation(b_sp[:], b_sp[:], act_ln, bias=1.0)
                nc.vector.tensor_mul(out=b_sp[:], in0=b_sp[:], in1=b_eps[:])
                nc.vector.tensor_add(out=bias_bf[:], in0=b_sp[:], in1=b_mu[:])

            # ---- W^T constructed in place. Per unit: one packed DMA,
            # Exp with fused uint8-rho dequant, Ln, eps-mul (bf16 2x),
            # fused (mu_i8 * scale) + sp_eps -> bf16 W^T block.
            WT = const.tile([P, KB, o_sh], bf16, tag="WT")
            for ui, (ub, us) in enumerate(units):
                pk = wcons.tile([P, us, PKB], u8, tag="pk")
                nc.gpsimd.dma_start(out=pk[:], in_=wpk[:, ub : ub + us, :])
                eps_v = pk[:, :, 0 : 2 * o_sh].bitcast(bf16)
                mu_v = pk[:, :, 2 * o_sh : 3 * o_sh].bitcast(i8)
                rho_v = pk[:, :, 3 * o_sh : 4 * o_sh]
                sp = wcons.tile([P, us, o_sh], bf16, tag="sp")
                nc.scalar.activation(sp[:], rho_v[:], act_exp, bias=rho_min, scale=rho_sc)
                nc.scalar.activation(sp[:], sp[:], act_ln, bias=1.0)
                nc.vector.tensor_mul(out=sp[:], in0=sp[:], in1=eps_v[:])
                nc.vector.scalar_tensor_tensor(
                    out=WT[:, ub : ub + us, :],
                    in0=mu_v[:],
                    scalar=mu_sc,
                    in1=sp[:],
                    op0=op_mult,
                    op1=op_add,
                )
                if ui == 2:
                    emit_bias_row()

            def body_tail(ps, bt):
                y_sb = yout.tile([P, o_sh], f32, tag="y_sb")
                nc.vector.tensor_add(out=y_sb[:], in0=ps[:], in1=bias_sb[:])
                nc.sync.dma_start(out=y[bt * P : (bt + 1) * P, :], in_=y_sb[:])

            # ---- first GROUP tiles run as a k-wavefront across PSUM
            # banks: wave w = tile i's matmul for k = w - i. Tile 0's
            # k=0 matmul needs only W^T block 0 + one x chunk.
            GROUP = min(7, BT)
            KC = KB // CH  # K-blocks per x chunk
            xts = []
            pss = []
            for bt in range(GROUP):
                xT = xin.tile([P, KB, P], bf16, tag="xT", name=f"xT_g{bt}")
                xts.append(xT)
                ps = psum_pool.tile([P, o_sh], f32, tag="ps", name=f"ps_g{bt}")
                pss.append(ps)

            # x chunk (i, c) is first read at wave i + c*KC; DMA in need
            # order with LOOKAHEAD waves of headroom so the packed-weight
            # DMAs keep their HBM share.
            chunks = sorted(
                ((i + c * KC, i, c) for i in range(GROUP) for c in range(CH))
            )

            def issue_chunks_through(wave):
                while chunks and chunks[0][0] <= wave:
                    _, i, c = chunks.pop(0)
                    ks = slice(c * KC, (c + 1) * KC)
                    nc.sync.dma_start(out=xts[i][:, ks, :], in_=x[i, :, ks, :])

            issue_chunks_through(LOOKAHEAD - 1)
            NWAVE = KB + GROUP - 1
            for w in range(NWAVE):
                issue_chunks_through(w + LOOKAHEAD)
                for i in range(GROUP):
                    k = w - i
                    if 0 <= k < KB:
                        nc.tensor.matmul(
                            pss[i][:],
                            lhsT=xts[i][:, k, :],
                            rhs=WT[:, k, :],
                            start=(k == 0),
                            stop=(k == KB - 1),
                        )
                if w == 12:
                    # bias broadcast: [128, o_sh] = ones.T @ bias_bf.
                    # Mid-stream so the in-order PE queue never blocks
                    # on the bias chain; ready long before 1st eviction.
                    bias_ps = bias_psum.tile(
                        [P, o_sh], f32, tag="bias_ps", name="bias_ps"
                    )
                    nc.tensor.matmul(bias_ps[:], lhsT=ones[:], rhs=bias_bf[:])
                    nc.vector.tensor_copy(out=bias_sb[:], in_=bias_ps[:])
                gi = w - (KB - 1)
                if 0 <= gi < GROUP:
                    body_tail(pss[gi], gi)

            # ---- remaining tiles stream one PSUM bank each
            for bt in range(GROUP, BT):
                xT = xin.tile([P, KB, P], bf16, tag="xT")
                nc.sync.dma_start(out=xT[:], in_=x[bt])
                ps = psum_pool.tile([P, o_sh], f32, tag="ps")
                for k in range(KB):
                    nc.tensor.matmul(
                        ps[:],
                        lhsT=xT[:, k, :],
                        rhs=WT[:, k, :],
                        start=(k == 0),
                        stop=(k == KB - 1),
                    )
                body_tail(ps, bt)

    # Skip bacc's pre-placed InstLoadActFuncSet: on large graphs walrus's
    # parallel-pass fork can separate the hoisted load from its activations
    # ("No Act func set exist for this instruction"); walrus's own lower_act
    # placement handles forked subgraphs correctly.
    nc.insert_act_table_loads = lambda: None
    nc.compile()
    return nc


def _prep_x(x):
    """[batch, in_f] fp32 -> bf16 tiled [BT, 128, KB, 128] with
    x_t[bt, pi, po, bi] = x[bt*128 + bi, po*128 + pi]."""
    batch, in_f = x.shape
    xb = x.astype(ml_dtypes.bfloat16)
    xb = xb.reshape(batch // P, P, in_f // P, P)  # [bt, bi, po, pi]
    return np.ascontiguousarray(xb.transpose(0, 3, 2, 1))  # [bt, pi, po, bi]


def _tile_w(w):
    """[o_sh, in_f] -> tiled [KB, 128, o_sh] with w_t[k, pi, o] = w[o, k*128 + pi]."""
    o_sh, in_f = w.shape
    return np.ascontiguousarray(w.T.reshape(in_f // P, P, o_sh))


def _prep_wpk(wmu, wrho, weps):
    """Pack eps (bf16 bytes), mu (int8 codes), rho (uint8 codes) into one
    uint8 [128, KB, 2048] tensor + the fp32 quant params [128, 4]."""
    eps_t = _tile_w(weps).astype(ml_dtypes.bfloat16)  # [KB, P, o]
    mu_t = _tile_w(wmu)
    rho_t = _tile_w(wrho)

    mu_sc = max(float(np.abs(mu_t).max()) / 127.0, 1e-30)
    mu_c = np.clip(np.round(mu_t / mu_sc), -127, 127).astype(np.int8)

    rmin = float(rho_t.min())
    rmax = float(rho_t.max())
    rho_sc = max((rmax - rmin) / 255.0, 1e-30)
    rho_c = np.clip(np.round((rho_t - rmin) / rho_sc), 0, 255).astype(np.uint8)

    kb, p, o = mu_t.shape
    pk = np.concatenate(
        [
            eps_t.view(np.uint8).reshape(kb, p, 2 * o),
            mu_c.view(np.uint8),
            rho_c,
        ],
        axis=2,
    )  # [KB, P, 4*o]
    qp = np.broadcast_to(
        np.array([rho_sc, rmin, mu_sc, 0.0], np.float32), (P, 4)
    ).copy()
    return np.ascontiguousarray(pk.transpose(1, 0, 2)), qp


def make_in_maps(x, weight_mu, weight_rho, bias_mu, bias_rho, weight_eps, bias_eps):
    o_sh = OUT_F // N_CORES
    x_t = _prep_x(np.asarray(x, dtype=np.float32))
    wmu = np.asarray(weight_mu, dtype=np.float32)
    wrho = np.asarray(weight_rho, dtype=np.float32)
    weps = np.asarray(weight_eps, dtype=np.float32)
    bmu = np.asarray(bias_mu, dtype=np.float32).reshape(1, -1)
    brho = np.asarray(bias_rho, dtype=np.float32).reshape(1, -1)
    beps = np.asarray(bias_eps, dtype=np.float32).reshape(1, -1)

    in_maps = []
    for c in range(N_CORES):
        rs = slice(c * o_sh, (c + 1) * o_sh)
        wpk, qp = _prep_wpk(wmu[rs], wrho[rs], weps[rs])
        in_maps.append(
            {
                "x_t": x_t,
                "wpk_t": wpk,
                "qp": qp,
                "bias_mu": np.ascontiguousarray(bmu[:, rs]),
                "bias_rho": np.ascontiguousarray(brho[:, rs]),
                "bias_eps": np.ascontiguousarray(beps[:, rs]),
            }
        )
    return in_maps


def kernel(x, weight_mu, weight_rho, bias_mu, bias_rho, weight_eps, bias_eps):
    o_sh = OUT_F // N_CORES
    key = (x.shape, o_sh)
    if key not in _NC_CACHE:
        _NC_CACHE[key] = build_nc(x.shape[0], x.shape[1], o_sh)
    nc = _NC_CACHE[key]

    in_maps = make_in_maps(
        x, weight_mu, weight_rho, bias_mu, bias_rho, weight_eps, bias_eps
    )
    res = run_bass_kernel_spmd(nc, in_maps, core_ids=list(range(N_CORES)))
    return np.concatenate([res.results[c]["y"] for c in range(N_CORES)], axis=1)
